# revision 8
# baseline (speedup 1.0000x reference)
"""Trainium2 Bass kernel: per-pixel channel shuffle + 3x3 conv (stride 1, pad 1).

Problem: x [32,256,56,56] f32, w [256,256,3,3] f32 (OIHW), perm [3136,256] i32;
out[b,:,h,w] = conv3x3(xs)[b,:,h,w] where xs[b,:,l] = x[b, perm[l,:], l].

Strategy (8 NeuronCores, data-parallel over batch, 4 batches/core):
  host: pre-transpose x to pixel-major bf16 [B,3136,256]; build inverse-perm
        int16 index table in the GPSIMD local_scatter layout; pre-transform w
        with the Winograd F(2,3) row filter G into 48 [128,128] bf16 lhsT
        tiles (V[r,n] = sum_m G[r,m] w[:,:,m,n]).
  device, per batch (pipelined; work on a quarter-image starts as soon as its
  scatter tiles land, so PE never idles on the shuffle):
    1. Per 112-pixel tile (2 image rows): DMA [l, c] tile (contiguous 512B
       runs), GPSIMD local_scatter applies each pixel's inverse channel
       permutation, PE-transpose -> [c, l] in PSUM, DVE-copy rows into one of
       4 zero-padded overlapping quarter images (16 rows x 58 each, 1-row
       halos).
    2. Row-Winograd conv per quarter: DVE computes T_r = B^T-row-combos of
       the quarter (4 tensors of 7 tile-rows x 58); for each oc-tile and r,
       accumulate 6 matmuls (3 col-shifts x 2 ic-tiles, N=406) into PSUM
       M_r; DVE computes z0 = M0+M1+M2, z1 = M1-M2-M3 (the A^T combos, which
       replace plain PSUM eviction) and DMAs the even/odd output rows out
       (scalar queue) with padding-stripping nested APs.
  This does 12 matmul passes per 2 output rows instead of 18 (2/3 PE work).
"""

import os
import sys
import types
import numpy as np

_STATE = {}
LAST_RESULT = None

B, C, H, W = 32, 256, 56, 56
HW = H * W
TL = 112          # pixels per scatter tile (2 image rows)
NT = 28           # scatter tiles per image
NQ = 4            # quarter images
QW = 58           # padded row width
QCT = 16 * QW + 2  # per-ic-tile span in a quarter tile (+2 overrun slack)
NG = 406          # matmul free size = 7 tile-rows x 58
TSPAN = 408       # per-r span in the T tensor (+2 shift slack)
N_CORES = 8
B_LOC = B // N_CORES


def _install_ntff_shim():
    # antenv.axon_hooks is absent in some images; provide it so trace=True
    # (BASS_TRACE=1) can capture NTFF profiles instead of crashing.
    name = "antenv.axon_hooks"
    if name in sys.modules:
        return
    try:
        import antenv  # noqa: F401

        m = types.ModuleType(name)
        m._hook = None
        m.set_axon_ntff_profile_hook = lambda h: setattr(m, "_hook", h)
        m.get_axon_ntff_profile_hook = lambda: m._hook
        sys.modules[name] = m
        setattr(sys.modules["antenv"], "axon_hooks", m)
        from trn_agent_boot.trn_boot import _ntff_profile_via_ctypes

        hook = _ntff_profile_via_ctypes("/opt/axon/libaxon_pjrt.so")
        if hook is not None:
            m.set_axon_ntff_profile_hook(hook)
    except Exception:
        pass


def _build_kernel():
    import concourse.bass as bass
    import concourse.mybir as mybir
    from concourse import bacc, tile
    from concourse.masks import make_identity
    from contextlib import ExitStack

    F32 = mybir.dt.float32
    BF16 = mybir.dt.bfloat16
    I16 = mybir.dt.int16

    nc = bacc.Bacc("TRN2", target_bir_lowering=False, debug=False, num_devices=N_CORES)

    xt = nc.dram_tensor("xt", [B_LOC, HW, C], BF16, kind="ExternalInput")
    wt = nc.dram_tensor("wt", [48, 128, 128], BF16, kind="ExternalInput")
    idxt = nc.dram_tensor("idxt", [128, NT * 256], I16, kind="ExternalInput")
    out = nc.dram_tensor("out", [B_LOC, C, HW], F32, kind="ExternalOutput")

    with tile.TileContext(nc) as tc, ExitStack() as ctx:
        const = ctx.enter_context(tc.tile_pool(name="const", bufs=1))
        ident = const.tile([128, 128], BF16)
        make_identity(nc, ident[:, :])

        # Pre-warm the GPSIMD local_scatter library (~6us IRAM load) with a
        # tiny all-ignored scatter so real scatters start ASAP.
        dd = const.tile([16, 256], BF16, name="dd", tag="dd")
        nc.vector.memset(dd[:, :], 0.0)
        didx = const.tile([16, 16], I16, name="didx", tag="didx")
        nc.vector.memset(didx[:, :], -1)
        dout = const.tile([16, 256], BF16, name="dout", tag="dout")
        nc.gpsimd.local_scatter(
            out_ap=dout[:, :],
            data_ap=dd[:, :],
            idxs_ap=didx[:, :],
            channels=16,
            num_elems=256,
            num_idxs=16,
        )

        idxsb = const.tile([128, NT * 256], I16)
        wsb = const.tile([128, 48 * 128], BF16)

        # 8 persistent quarter tiles (4 quarters x double buffer across
        # batches); zero only the padding borders (cols 0/57, top/bottom halo
        # rows, overrun slack) once -- interiors are overwritten every batch.
        qts = []
        for qi in range(2 * NQ):
            qt = const.tile([128, 2 * QCT], BF16, name=f"qt{qi}", tag=f"qt{qi}")
            for ct in range(2):
                base = ct * QCT
                rows = qt[:, base : base + 16 * QW].rearrange(
                    "p (r x) -> p r x", r=16
                )
                nc.vector.memset(rows[:, :, 0:1], 0.0)
                nc.vector.memset(rows[:, :, 57:58], 0.0)
                nc.vector.memset(qt[:, base + 16 * QW : base + QCT], 0.0)
                if qi % NQ == 0:
                    nc.vector.memset(qt[:, base : base + QW], 0.0)
                if qi % NQ == NQ - 1:
                    nc.vector.memset(qt[:, base + 15 * QW : base + 16 * QW], 0.0)
            qts.append(qt)

        # 4 persistent T tensors (2 ct x double buffer over quarters); only
        # the 2-elem shift slack after each r-span needs zeroing once.
        tts = []
        for ti in range(4):
            tt = const.tile([128, 4 * TSPAN], BF16, name=f"tt{ti}", tag=f"tt{ti}")
            for r in range(4):
                nc.vector.memset(tt[:, r * TSPAN + NG : (r + 1) * TSPAN], 0.0)
            tts.append(tt)

        # input DMAs all on the sync queue, interleaved by the time they are
        # needed; weights after the first few x tiles
        nc.sync.dma_start(out=idxsb[:, 0:512], in_=idxt[:, 0:512])

        xin_pool = ctx.enter_context(tc.tile_pool(name="xin", bufs=12))
        sout_pool = ctx.enter_context(tc.tile_pool(name="sout", bufs=12))
        tmp_pool = ctx.enter_context(tc.tile_pool(name="tmp", bufs=6))
        outst_pool = ctx.enter_context(tc.tile_pool(name="outst", bufs=4))
        psb_pool = ctx.enter_context(tc.tile_pool(name="psb", bufs=1, space="PSUM"))
        psbig = psb_pool.tile([128, 1024], BF16)  # 1 bank, 4 ps2 slots of 224
        mpsum_pool = ctx.enter_context(tc.tile_pool(name="mpsum", bufs=7, space="PSUM"))

        def shuffle_tile(b, t):
            # 2 image rows (2t, 2t+1) -> scatter -> [c, l] -> quarter tiles
            xin = xin_pool.tile([128, 256], BF16, name="xin", tag="xin")
            nc.sync.dma_start(
                out=xin[0:TL, :], in_=xt[b, t * TL : (t + 1) * TL, :]
            )
            if b == 0 and t == 0:
                nc.sync.dma_start(out=idxsb[:, 512 : 14 * 256], in_=idxt[:, 512 : 14 * 256])
            if b == 0 and t == 1:
                nc.sync.dma_start(
                    out=wsb[:, :],
                    in_=bass.AP(wt, 0, [[128, 128], [128 * 128, 48], [1, 128]]),
                )
            if b == 0 and t == 2:
                nc.sync.dma_start(
                    out=idxsb[:, 14 * 256 :], in_=idxt[:, 14 * 256 :]
                )
            sout = sout_pool.tile([128, 256], BF16, name="sout", tag="sout")
            nc.gpsimd.local_scatter(
                out_ap=sout[0:TL, :],
                data_ap=xin[0:TL, :],
                idxs_ap=idxsb[0:TL, t * 256 : (t + 1) * 256],
                channels=TL,
                num_elems=256,
                num_idxs=256,
            )
            sl = (t % 4) * 224
            ps2 = psbig[:, sl : sl + 224]
            for ct in range(2):
                nc.tensor.transpose(
                    ps2[:, ct * TL : (ct + 1) * TL],
                    sout[0:TL, ct * 128 : (ct + 1) * 128],
                    ident[0:TL, 0:TL],
                )
            q, i = divmod(t, 7)  # main quarter, local 2-row index
            qt = qts[(b % 2) * NQ + q]
            for ct in range(2):
                # rows 2t, 2t+1 = quarter-local rows 2i+1, 2i+2
                dst = qt[
                    :, ct * QCT + (2 * i + 1) * QW : ct * QCT + (2 * i + 3) * QW
                ].rearrange("p (r x) -> p r x", r=2)[:, :, 1:57]
                src = ps2[:, ct * TL : (ct + 1) * TL].rearrange(
                    "p (r x) -> p r x", r=2
                )
                nc.vector.tensor_copy(dst, src)
                if i == 0 and q > 0:
                    # row 2t is also the trailing halo (local row 15) of q-1
                    qprev = qts[(b % 2) * NQ + q - 1]
                    nc.vector.tensor_copy(
                        qprev[:, ct * QCT + 15 * QW + 1 : ct * QCT + 15 * QW + 57],
                        ps2[:, ct * TL : ct * TL + 56],
                    )
                if i == 6 and q < NQ - 1:
                    # row 2t+1 is also the leading halo (local row 0) of q+1
                    qnext = qts[(b % 2) * NQ + q + 1]
                    nc.vector.tensor_copy(
                        qnext[:, ct * QCT + 1 : ct * QCT + 57],
                        ps2[:, ct * TL + 56 : ct * TL + 112],
                    )

        # B^T row combos: t0 = d0-d2, t1 = d1+d2, t2 = d2-d1, t3 = d1-d3
        # where d_rho = quarter-local row 2*tau + rho.
        TDEFS = [  # (in0 rho, in1 rho, is_sub)
            (0, 2, True),
            (1, 2, False),
            (2, 1, True),
            (1, 3, True),
        ]

        def transform_half(b, q, h):
            tau0, ntau = (0, 3) if h == 0 else (3, 4)
            qt = qts[(b % 2) * NQ + q]
            for ct in range(2):
                rows5 = qt[:, ct * QCT : ct * QCT + 16 * QW].rearrange(
                    "p (a u x) -> p a u x", a=8, u=2
                )
                tt = tts[(q % 2) * 2 + ct]
                for r, (r0, r1, sub) in enumerate(TDEFS):
                    dst = tt[
                        :, r * TSPAN + tau0 * QW : r * TSPAN + (tau0 + ntau) * QW
                    ].rearrange("p (a u x) -> p a u x", a=ntau, u=1)
                    in0 = rows5[:, tau0 + r0 // 2 : tau0 + r0 // 2 + ntau, r0 % 2 : r0 % 2 + 1, :]
                    in1 = rows5[:, tau0 + r1 // 2 : tau0 + r1 // 2 + ntau, r1 % 2 : r1 % 2 + 1, :]
                    if sub:
                        nc.vector.tensor_sub(dst, in0, in1)
                    else:
                        nc.vector.tensor_add(dst, in0, in1)

        def gemm_quarter(b, q):
            for oct in range(2):
                out5 = out[b, oct * 128 : (oct + 1) * 128, :].rearrange(
                    "p (a u x) -> p a u x", a=28, u=2
                )
                Ms = []
                e0 = e1 = s01 = d12 = None
                for r in range(4):
                    mp = mpsum_pool.tile([128, NG], F32, name="mp", tag="mp")
                    Ms.append(mp)
                    for ct in range(2):
                        tt = tts[(q % 2) * 2 + ct]
                        for n in range(3):
                            widx = ((r * 3 + n) * 2 + ct) * 2 + oct
                            nc.tensor.matmul(
                                mp[:, :],
                                lhsT=wsb[:, widx * 128 : (widx + 1) * 128],
                                rhs=tt[:, r * TSPAN + n : r * TSPAN + n + NG],
                                start=(ct == 0 and n == 0),
                                stop=(ct == 1 and n == 2),
                            )
                    # A^T combos (z0 = M0+M1+M2 even rows, z1 = M1-M2-M3 odd
                    # rows), interleaved so PSUM banks free early. DVE can
                    # read only one PSUM operand per op, so M0/M1 are staged
                    # to SBUF via the otherwise-idle scalar engine.
                    if r == 0:
                        e0 = tmp_pool.tile([128, NG], F32, name="e0", tag="tmp")
                        nc.scalar.copy(e0[:, :], Ms[0][:, :])
                    elif r == 1:
                        e1 = tmp_pool.tile([128, NG], F32, name="e1", tag="tmp")
                        nc.scalar.copy(e1[:, :], Ms[1][:, :])
                        s01 = tmp_pool.tile([128, NG], F32, name="s01", tag="tmp")
                        nc.vector.tensor_add(s01[:, :], e0[:, :], Ms[1][:, :])
                    elif r == 2:
                        z0 = outst_pool.tile([128, NG], F32, name="z0", tag="ost")
                        nc.vector.tensor_add(z0[:, :], s01[:, :], Ms[2][:, :])
                        nc.scalar.dma_start(
                            out=out5[:, 7 * q : 7 * q + 7, 0:1, :],
                            in_=z0[:, :].rearrange("p (a u x) -> p a u x", a=7, u=1)[
                                :, :, :, 0:56
                            ],
                        )
                        d12 = tmp_pool.tile([128, NG], F32, name="d12", tag="tmp")
                        nc.vector.tensor_sub(d12[:, :], e1[:, :], Ms[2][:, :])
                    elif r == 3:
                        z1 = outst_pool.tile([128, NG], F32, name="z1", tag="ost")
                        nc.vector.tensor_sub(z1[:, :], d12[:, :], Ms[3][:, :])
                        nc.scalar.dma_start(
                            out=out5[:, 7 * q : 7 * q + 7, 1:2, :],
                            in_=z1[:, :].rearrange("p (a u x) -> p a u x", a=7, u=1)[
                                :, :, :, 0:56
                            ],
                        )

        for b in range(B_LOC):
            if b > 0:
                transform_half(b - 1, 3, 1)
                gemm_quarter(b - 1, 3)
            for t in range(NT):
                shuffle_tile(b, t)
                if t % 7 == 4:
                    transform_half(b, t // 7, 0)
                if t in (8, 15, 22):
                    q = (t - 8) // 7
                    transform_half(b, q, 1)
                    gemm_quarter(b, q)
        transform_half(B_LOC - 1, 3, 1)
        gemm_quarter(B_LOC - 1, 3)

    nc.compile()
    return nc


def _host_prep(x, w, perm):
    import ml_dtypes

    # pixel-major bf16: [B, HW, C]
    xf = np.ascontiguousarray(
        x.reshape(B, C, HW).transpose(0, 2, 1)
    ).astype(ml_dtypes.bfloat16)

    # Winograd F(2,3) row-filter transform: V[r,n] = sum_m G[r,m] w[:,:,m,n]
    wf = np.asarray(w, dtype=np.float64)
    G = np.array([[1, 0, 0], [0.5, 0.5, 0.5], [0.5, -0.5, 0.5], [0, 0, 1]])
    V = np.einsum("rm,ocmn->rnoc", G, wf)  # [4, 3, OC, C]
    wt = np.empty((48, 128, 128), dtype=ml_dtypes.bfloat16)
    for r in range(4):
        for n in range(3):
            for ct in range(2):
                for oct in range(2):
                    i = ((r * 3 + n) * 2 + ct) * 2 + oct
                    wt[i] = (
                        V[r, n, oct * 128 : (oct + 1) * 128, ct * 128 : (ct + 1) * 128]
                        .T.astype(ml_dtypes.bfloat16)
                    )

    iperm = np.empty((HW, C), dtype=np.int16)
    np.put_along_axis(
        iperm, perm.astype(np.int64), np.arange(C, dtype=np.int16)[None, :], axis=1
    )
    idxt = np.zeros((128, NT * 256), dtype=np.int16)
    for t in range(NT):
        idxt[0:TL, t * 256 : (t + 1) * 256] = iperm[t * TL : t * TL + TL, :]

    in_maps = []
    for cidx in range(N_CORES):
        in_maps.append(
            {
                "xt": np.ascontiguousarray(xf[cidx * B_LOC : (cidx + 1) * B_LOC]),
                "wt": wt,
                "idxt": idxt,
            }
        )
    return in_maps


def kernel(x, w, perm):
    global LAST_RESULT
    _install_ntff_shim()
    from concourse.bass_utils import run_bass_kernel_spmd

    x = np.asarray(x, dtype=np.float32)
    w = np.asarray(w, dtype=np.float32)
    perm = np.asarray(perm)

    if "nc" not in _STATE:
        _STATE["nc"] = _build_kernel()
    nc = _STATE["nc"]

    in_maps = _host_prep(x, w, perm)
    res = run_bass_kernel_spmd(nc, in_maps, core_ids=list(range(N_CORES)))
    LAST_RESULT = res
    out = np.concatenate(
        [r["out"].reshape(B_LOC, C, H, W) for r in res.results], axis=0
    )
    return out.astype(np.float32)
